# revision 6
# baseline (speedup 1.0000x reference)
"""Trainium2 Bass kernel for nn_Direction: out = input @ qr(weight + 1e-8).Q^T.

input: (262144, 20) fp32, weight: (512, 20) fp32 -> out: (262144, 512) fp32.

Strategy (data-parallel over batch, 8 cores; memory/DMA-bound target):
  - Host: QR of the tiny 512x20 weight (LAPACK); Q^T cast to fp16 and
    replicated into the four 32-partition quadrant blocks. The input is cast
    to fp16 and pre-TRANSPOSED on host into x^T layout (m on partitions,
    rows on free), so the device does no transposes at all.
  - Tolerance is rel_err < 2e-2 vs max|out|; a single fp16 matmul pass with
    fp16 output is ~1e-3 -- so the output is written to HBM as fp16 (halving
    the dominant output traffic vs fp32) and converted to fp32 on host.
  - Device per core (32768 rows = 4 quadrants x 64 j-steps x 128 rows):
    each j-step issues four K=20 matmuls, one per PE row-quadrant
    (tile_position=(32k,0)), producing four [128,512] fp32 PSUM tiles that
    are copied (DVE/Pool/ACT rotating) into per-quadrant fp16 SBUF slabs,
    flushed every FJ j-steps as [128, FJ*512] DMAs on the sync queue.
  - Per-core HBM traffic: 1.3 MB in + 33.6 MB out (~35 MB vs 71.3 MB for
    the fp32 baseline).
"""

import numpy as np

B = 262144
M = 20
F = 512
NCORES = 8
BL = B // NCORES           # 32768 rows per core
NQ = 4                     # PE row-quadrants
QROWS = BL // NQ           # 8192 rows per quadrant
NJ = QROWS // 128          # 64 j-steps of 128 rows
# Graded flush sizes (in j-steps): small at the start so the output-DMA
# pipeline ramps immediately, small at the end so the post-compute drain is
# tiny, 4-step (512KB) pieces in steady state.
FLUSH_SIZES = [1, 1, 2] + [4] * 14 + [2, 1, 1]
assert sum(FLUSH_SIZES) == NJ
# Input chunk sizes (in j-steps): small first chunk so matmuls start early.
CHUNK_SIZES = [4, 12, 16, 16, 16]
assert sum(CHUNK_SIZES) == NJ

_CACHE = {}


def _build_nc():
    import concourse.bass as bass
    import concourse.tile as tile
    from concourse import bacc, mybir

    f32 = mybir.dt.float32
    f16 = mybir.dt.float16
    COPY = mybir.ActivationFunctionType.Copy

    nc = bacc.Bacc(None, target_bir_lowering=False, debug=False)
    xt = nc.dram_tensor("xt", [NQ, M, QROWS], f16, kind="ExternalInput")
    q = nc.dram_tensor("q", [128, F], f16, kind="ExternalInput")
    out = nc.dram_tensor("out", [NQ, 128, NJ * F], f16, kind="ExternalOutput")

    with tile.TileContext(nc) as tc:
        with (
            tc.tile_pool(name="const", bufs=1) as cpool,
            tc.tile_pool(name="xin", bufs=2) as xin_pool,
            tc.tile_pool(name="osl", bufs=3 * NQ) as osl_pool,
            tc.tile_pool(name="ps", bufs=8, space=bass.MemorySpace.PSUM) as ps_pool,
        ):
            q_t = cpool.tile([128, F], f16, tag="q")
            nc.sync.dma_start(q_t[:], q[:])

            # per-j lookup tables from the chunk / flush plans
            chunk_of_j, chunk_start = [], []
            for c, sz in enumerate(CHUNK_SIZES):
                chunk_of_j += [c] * sz
                chunk_start.append(sum(CHUNK_SIZES[:c]))
            flush_of_j, flush_start = [], []
            for fidx, sz in enumerate(FLUSH_SIZES):
                flush_of_j += [fidx] * sz
                flush_start.append(sum(FLUSH_SIZES[:fidx]))

            xt_tiles = [None] * len(CHUNK_SIZES)

            def load_chunk(c):
                c0, sz = chunk_start[c] * 128, CHUNK_SIZES[c] * 128
                t = xin_pool.tile([128, sz], f16, name=f"xc_{c}", tag=f"xc_{CHUNK_SIZES[c]}")
                for k in range(NQ):
                    nc.scalar.dma_start(
                        t[32 * k:32 * k + M, :], xt[k][:, c0:c0 + sz]
                    )
                xt_tiles[c] = t

            load_chunk(0)
            osl = [None] * NQ

            for j in range(NJ):
                c = chunk_of_j[j]
                jc = j - chunk_start[c]
                if jc == 0 and c + 1 < len(CHUNK_SIZES):
                    load_chunk(c + 1)
                fidx = flush_of_j[j]
                jf = j - flush_start[fidx]
                if jf == 0:
                    for k in range(NQ):
                        osl[k] = osl_pool.tile(
                            [128, 4 * F], f16, name=f"os_{j}_{k}", tag="os"
                        )
                xc = xt_tiles[c]
                for k in range(NQ):
                    ps = ps_pool.tile([128, F], f32, name=f"ps_{j}_{k}", tag="ps")
                    nc.tensor.matmul(
                        ps[:],
                        xc[32 * k:32 * k + M, jc * 128:(jc + 1) * 128],
                        q_t[32 * k:32 * k + M, :],
                        start=True, stop=True,
                        tile_position=(32 * k, 0),
                    )
                    dst = osl[k][:, jf * F:(jf + 1) * F]
                    # GPSIMD/Pool cannot read PSUM on TRN2; alternate DVE/ACT.
                    if k % 2 == 0:
                        nc.vector.tensor_copy(dst, ps[:])
                    else:
                        nc.scalar.activation(dst, ps[:], COPY)
                if jf == FLUSH_SIZES[fidx] - 1:
                    f0 = flush_start[fidx]
                    for k in range(NQ):
                        nc.sync.dma_start(
                            out[k][:, f0 * F:(j + 1) * F],
                            osl[k][:, 0:(j + 1 - f0) * F],
                        )

    nc.compile()
    return nc


def _get_nc():
    if "nc" not in _CACHE:
        _CACHE["nc"] = _build_nc()
    return _CACHE["nc"]


def _prep_inputs(input, weight):
    w = weight.astype(np.float32) + np.float32(1e-8)
    qm, _ = np.linalg.qr(w)                     # reduced: (512, 20)
    qt16 = np.ascontiguousarray(qm.T).astype(np.float16)   # (20, 512)
    qrep = np.zeros((128, F), dtype=np.float16)
    for k in range(NQ):
        qrep[32 * k:32 * k + M] = qt16

    # x^T per core/quadrant: xt[core][k][m][j*128+p] = x[core*BL + k*QROWS + j*128 + p, m]
    x16 = np.asarray(input, dtype=np.float16)
    xt = np.ascontiguousarray(
        x16.reshape(NCORES, NQ, QROWS, M).transpose(0, 1, 3, 2)
    )
    return [{"xt": xt[c], "q": qrep} for c in range(NCORES)]


def _unpack(res):
    full = np.empty((B, F), dtype=np.float32)
    for c in range(NCORES):
        o = res.results[c]["out"]               # (NQ, 128, NJ*F) fp16
        o = o.reshape(NQ, 128, NJ, F).transpose(0, 2, 1, 3).reshape(BL, F)
        full[c * BL:(c + 1) * BL] = o
    return full


def _run(input, weight, trace=False):
    from concourse.bass_utils import run_bass_kernel_spmd

    nc = _get_nc()
    in_maps = _prep_inputs(input, weight)
    res = run_bass_kernel_spmd(nc, in_maps, list(range(NCORES)), trace=trace)
    return _unpack(res), res


def kernel(input, weight):
    # If BASS_TRACE is set externally but the NTFF hook shim (antenv.axon_hooks)
    # isn't importable, run_bass_kernel_spmd's trace path would crash; force
    # the no-trace path in that case.
    try:
        import antenv.axon_hooks  # noqa: F401
    except ImportError:
        import os
        os.environ["BASS_NEVER_TRACE"] = "1"
    out, _ = _run(input, weight, trace=False)
    return out


# revision 7
# speedup vs baseline: 1.0402x; 1.0402x over previous
"""Trainium2 Bass kernel for nn_Direction: out = input @ qr(weight + 1e-8).Q^T.

input: (262144, 20) fp32, weight: (512, 20) fp32 -> out: (262144, 512) fp32.

Strategy (data-parallel over batch, 8 cores; memory/DMA-bound target):
  - Host: QR of the tiny 512x20 weight (LAPACK); Q^T cast to fp16 and
    replicated into the four 32-partition quadrant blocks. The input is cast
    to fp16 and pre-TRANSPOSED on host into x^T layout (m on partitions,
    rows on free), so the device does no transposes at all.
  - Tolerance is rel_err < 2e-2 vs max|out|; a single fp16 matmul pass with
    fp16 output is ~6e-4 -- so the output is written to HBM as fp16 (halving
    the dominant output traffic vs fp32) and converted to fp32 on host.
  - Device per core (32768 rows = 4 quadrants x 64 j-steps x 128 rows):
    each j-step issues four K=20 matmuls, one per PE row-quadrant
    (tile_position=(32k,0)). Quadrant pairs (0,1) and (2,3) write the two
    banks of a shared 2-bank PSUM tile, so each engine (DVE for pair01,
    ACT for pair23) drains a whole pair with ONE 1024-wide fp32->fp16 copy
    per j-step (amortizes the ~160-260ns per-instruction overhead; PSUM
    reads stream at ~1 elem/lane/cycle and are the production-rate wall).
  - Output DRAM is pair-major ([pair][partition][j*2*F]) so flushes are
    fully contiguous [128, FJ*2*F] DMAs on the sync queue; the host undoes
    the pair interleave for free during the fp16->fp32 convert.
  - Per-core HBM traffic: 1.3 MB in + 33.6 MB out (~35 MB vs 71.3 MB for
    the fp32 baseline). Steady-state DMA ceiling is ~408 GB/s (16 engines
    x ~25.5 GB/s), so graded flush sizes keep the ramp short.
"""

import numpy as np

B = 262144
M = 20
F = 512
NCORES = 8
BL = B // NCORES           # 32768 rows per core
NQ = 4                     # PE row-quadrants
NPAIR = 2                  # quadrant pairs (2 banks / copy engine)
QROWS = BL // NQ           # 8192 rows per quadrant
NJ = QROWS // 128          # 64 j-steps of 128 rows
# Graded flush sizes (in j-steps): small at the start so the output-DMA
# pipeline ramps immediately, 4-step (1MB/pair) pieces in steady state.
FLUSH_SIZES = [1, 1, 2] + [4] * 15
assert sum(FLUSH_SIZES) == NJ
# Input chunk sizes (in j-steps): small first chunk so matmuls start early.
CHUNK_SIZES = [4, 12, 16, 16, 16]
assert sum(CHUNK_SIZES) == NJ

_CACHE = {}


def _build_nc():
    import concourse.bass as bass
    import concourse.tile as tile
    from concourse import bacc, mybir

    f32 = mybir.dt.float32
    f16 = mybir.dt.float16
    COPY = mybir.ActivationFunctionType.Copy

    nc = bacc.Bacc(None, target_bir_lowering=False, debug=False)
    xt = nc.dram_tensor("xt", [NQ, M, QROWS], f16, kind="ExternalInput")
    q = nc.dram_tensor("q", [128, F], f16, kind="ExternalInput")
    out = nc.dram_tensor("out", [NPAIR, 128, NJ * 2 * F], f16, kind="ExternalOutput")

    with tile.TileContext(nc) as tc:
        with (
            tc.tile_pool(name="const", bufs=1) as cpool,
            tc.tile_pool(name="xin", bufs=2) as xin_pool,
            tc.tile_pool(name="osl", bufs=3 * NPAIR) as osl_pool,
            tc.tile_pool(name="ps", bufs=4, space=bass.MemorySpace.PSUM) as ps_pool,
        ):
            q_t = cpool.tile([128, F], f16, tag="q")
            nc.sync.dma_start(q_t[:], q[:])

            # per-j lookup tables from the chunk / flush plans
            chunk_of_j, chunk_start = [], []
            for c, sz in enumerate(CHUNK_SIZES):
                chunk_of_j += [c] * sz
                chunk_start.append(sum(CHUNK_SIZES[:c]))
            flush_of_j, flush_start = [], []
            for fidx, sz in enumerate(FLUSH_SIZES):
                flush_of_j += [fidx] * sz
                flush_start.append(sum(FLUSH_SIZES[:fidx]))

            xt_tiles = [None] * len(CHUNK_SIZES)

            def load_chunk(c):
                c0, sz = chunk_start[c] * 128, CHUNK_SIZES[c] * 128
                t = xin_pool.tile([128, sz], f16, name=f"xc_{c}", tag=f"xc_{CHUNK_SIZES[c]}")
                for k in range(NQ):
                    nc.scalar.dma_start(
                        t[32 * k:32 * k + M, :], xt[k][:, c0:c0 + sz]
                    )
                xt_tiles[c] = t

            load_chunk(0)
            osl = [None] * NPAIR

            for j in range(NJ):
                c = chunk_of_j[j]
                jc = j - chunk_start[c]
                if jc == 0 and c + 1 < len(CHUNK_SIZES):
                    load_chunk(c + 1)
                fidx = flush_of_j[j]
                jf = j - flush_start[fidx]
                if jf == 0:
                    for p in range(NPAIR):
                        osl[p] = osl_pool.tile(
                            [128, 4 * 2 * F], f16, name=f"os_{j}_{p}", tag="os"
                        )
                xc = xt_tiles[c]
                for p in range(NPAIR):
                    ps = ps_pool.tile([128, 2 * F], f32, name=f"ps_{j}_{p}", tag="ps")
                    for kk in range(2):
                        k = 2 * p + kk
                        nc.tensor.matmul(
                            ps[:, kk * F:(kk + 1) * F],
                            xc[32 * k:32 * k + M, jc * 128:(jc + 1) * 128],
                            q_t[32 * k:32 * k + M, :],
                            start=True, stop=True,
                            tile_position=(32 * k, 0),
                        )
                    dst = osl[p][:, jf * 2 * F:(jf + 1) * 2 * F]
                    # one 2-bank (1024-wide) PSUM drain per engine per j-step;
                    # GPSIMD/Pool cannot read PSUM on TRN2.
                    if p == 0:
                        nc.vector.tensor_copy(dst, ps[:])
                    else:
                        nc.scalar.activation(dst, ps[:], COPY)
                if jf == FLUSH_SIZES[fidx] - 1:
                    f0 = flush_start[fidx]
                    for p in range(NPAIR):
                        nc.sync.dma_start(
                            out[p][:, f0 * 2 * F:(j + 1) * 2 * F],
                            osl[p][:, 0:(j + 1 - f0) * 2 * F],
                        )

    nc.compile()
    return nc


def _get_nc():
    if "nc" not in _CACHE:
        _CACHE["nc"] = _build_nc()
    return _CACHE["nc"]


def _prep_inputs(input, weight):
    w = weight.astype(np.float32) + np.float32(1e-8)
    qm, _ = np.linalg.qr(w)                     # reduced: (512, 20)
    qt16 = np.ascontiguousarray(qm.T).astype(np.float16)   # (20, 512)
    qrep = np.zeros((128, F), dtype=np.float16)
    for k in range(NQ):
        qrep[32 * k:32 * k + M] = qt16

    # x^T per core/quadrant: xt[core][k][m][j*128+p] = x[core*BL + k*QROWS + j*128 + p, m]
    x16 = np.asarray(input, dtype=np.float16)
    xtp = np.ascontiguousarray(
        x16.reshape(NCORES, NQ, QROWS, M).transpose(0, 1, 3, 2)
    )
    return [{"xt": xtp[c], "q": qrep} for c in range(NCORES)]


def _unpack(res):
    full = np.empty((B, F), dtype=np.float32)
    for c in range(NCORES):
        o = res.results[c]["out"]               # (NPAIR, 128, NJ*2*F) fp16
        # out[pair][p][j*2F + kk*F + f] = row (2*pair+kk)*QROWS + j*128 + p
        o = o.reshape(NPAIR, 128, NJ, 2, F).transpose(0, 3, 2, 1, 4).reshape(BL, F)
        full[c * BL:(c + 1) * BL] = o
    return full


def _run(input, weight, trace=False):
    from concourse.bass_utils import run_bass_kernel_spmd

    nc = _get_nc()
    in_maps = _prep_inputs(input, weight)
    res = run_bass_kernel_spmd(nc, in_maps, list(range(NCORES)), trace=trace)
    return _unpack(res), res


def kernel(input, weight):
    # If BASS_TRACE is set externally but the NTFF hook shim (antenv.axon_hooks)
    # isn't importable, run_bass_kernel_spmd's trace path would crash; force
    # the no-trace path in that case.
    try:
        import antenv.axon_hooks  # noqa: F401
    except ImportError:
        import os
        os.environ["BASS_NEVER_TRACE"] = "1"
    out, _ = _run(input, weight, trace=False)
    return out
